# revision 25
# baseline (speedup 1.0000x reference)
"""CrossAttention Trainium2 Bass kernel (v2: bf16, no device transposes).

Problem (hardcoded shapes): B=8, N=S=1024, DIM=1024, H=16, DH=64.
  q = (queries @ Wq.T).reshape(B, H, N, DH)   # direct reshape, NOT a head transpose
  attn = softmax(q @ k^T * DH**-0.5); out = attn @ v
  out = out.transpose(0,2,1,3).reshape(B,N,H*DH) @ Wo.T + bo
Sharding: data-parallel over batch B (one batch element per core, weights
replicated, no collectives).

Host marshalling: inputs arrive transposed and bf16-cast (xqT = queries[i].T,
wqT = Wq.T, ... woT = Wo.T), so the device never runs a PE transpose.

Device per core (X one batch):
  Stage A: QnT = (X@Wq.T).T = Wq@X.T and KnT likewise ([j, n] orientation),
  Vn = X@Wv.T ([n, j]); all bf16, bounced to DRAM so stage B's torch-reshape
  head views become strided (128B-chunk) DMA gathers:
    head h of q as [d, n'] is QnT[g*64+d, h*64+r] with n' = r*16+g
    head h of v as [s', d] is Vn[h*64 + s'//16, (s'%16)*64 + d]
  Stage B per head: scores S_T[s', n'] = kT.T @ qT per 128-row s' block,
  exp on ScalarE (bf16 out), attn@v accumulates po[0:64] with an extra
  all-ones v column producing the softmax denominator on po row 64;
  reciprocal_approx_fast + DRAM-broadcast + tensor_mul divides.
  Stage C: out = outT.T @ woT + bo.
"""

import numpy as np

import concourse.bass as bass
import concourse.mybir as mybir
import concourse.tile as tile
from concourse import bacc

B, N, S, DIM, H, DH = 8, 1024, 1024, 1024, 16, 64
SCALE = DH**-0.5
P = 128
F32 = mybir.dt.float32
BF16 = mybir.dt.bfloat16
AF = mybir.ActivationFunctionType


def build(debug: bool = False) -> bacc.Bacc:
    nc = bacc.Bacc("TRN2", target_bir_lowering=False, debug=debug, num_devices=B)

    xqT = nc.dram_tensor("xqT", [DIM, N], BF16, kind="ExternalInput")
    xkT = nc.dram_tensor("xkT", [DIM, S], BF16, kind="ExternalInput")
    xvT = nc.dram_tensor("xvT", [DIM, S], BF16, kind="ExternalInput")
    wqT = nc.dram_tensor("wqT", [DIM, H * DH], BF16, kind="ExternalInput")
    wkT = nc.dram_tensor("wkT", [DIM, H * DH], BF16, kind="ExternalInput")
    wvT = nc.dram_tensor("wvT", [DIM, H * DH], BF16, kind="ExternalInput")
    woT = nc.dram_tensor("woT", [H * DH, DIM], BF16, kind="ExternalInput")
    bo = nc.dram_tensor("bo", [1, DIM], F32, kind="ExternalInput")
    out = nc.dram_tensor("out", [N, DIM], F32, kind="ExternalOutput")

    with tile.TileContext(nc) as tc:
        with (
            tc.tile_pool(name="const", bufs=1) as const,
            tc.tile_pool(name="persist", bufs=1) as persist,
            tc.tile_pool(name="stg", bufs=3) as stg_pool,
            tc.tile_pool(name="heads", bufs=2) as heads,
            tc.tile_pool(name="work", bufs=3) as work,
            tc.tile_pool(name="sm", bufs=2) as sm,
            tc.tile_pool(name="mm_psum", bufs=2, space="PSUM") as mm_psum,
            tc.tile_pool(name="o_psum", bufs=2, space="PSUM") as o_psum,
            tc.tile_pool(name="dram", bufs=1, space="DRAM") as dram,
        ):
            ones8 = const.tile([P, 8, 1], BF16)
            nc.vector.memset(ones8, 1.0)
            bo_bc = persist.tile([P, DIM], F32)
            nc.sync.dma_start(bo_bc, bo[:].to_broadcast((P, DIM)))

            # SBUF-resident operands, [c (or j), ck, free] with c = ck*128 + p
            xq_sb = persist.tile([P, 8, N], BF16, name="xq_sb")
            xk_sb = persist.tile([P, 8, S], BF16, name="xk_sb")
            xv_sb = persist.tile([P, 8, S], BF16, name="xv_sb")
            wq_sb = persist.tile([P, 8, H * DH], BF16, name="wq_sb")
            wk_sb = persist.tile([P, 8, H * DH], BF16, name="wk_sb")
            wv_sb = persist.tile([P, 8, H * DH], BF16, name="wv_sb")
            wo_sb = persist.tile([P, 8, DIM], BF16, name="wo_sb")
            outT = persist.tile([P, 8, N], BF16, name="outT")

            for src, dst in (
                (xqT, xq_sb), (xkT, xk_sb), (xvT, xv_sb),
                (wqT, wq_sb), (wkT, wk_sb), (wvT, wv_sb), (woT, wo_sb),
            ):
                v = src[:].rearrange("(ck c) f -> c ck f", c=P)
                for i in range(4):
                    nc.sync.dma_start(dst[:, 2 * i : 2 * i + 2, :], v[:, 2 * i : 2 * i + 2, :])

            QnT = dram.tile([H * DH, N], BF16, name="QnT")
            KnT = dram.tile([H * DH, S], BF16, name="KnT")
            Vn = dram.tile([S, H * DH], BF16, name="Vn")

            # ---- Stage A: projections (no transposes; operands pre-transposed) ----
            # QnT[j, n] = sum_c Wq[j, c] X[n, c] : lhsT = wq_sb (stationary), rhs = xq_sb
            # Vn[n, j]  = sum_c X[n, c] Wv[j, c] : lhsT = xv_sb, rhs = wv_sb
            def gemm(lhs_sb, rhs_sb, dst, tag, trange, jsrange):
                for t in trange:
                    for js in jsrange:
                        pp = mm_psum.tile(
                            [P, 512], F32, tag="mm", name=f"pp_{tag}_{t}_{js}"
                        )
                        for ck in range(8):
                            nc.tensor.matmul(
                                pp,
                                lhs_sb[:, ck, t * P : (t + 1) * P],
                                rhs_sb[:, ck, js * 512 : (js + 1) * 512],
                                start=(ck == 0),
                                stop=(ck == 7),
                            )
                        stg = stg_pool.tile(
                            [P, 512], BF16, tag="stg", name=f"stg_{tag}_{t}_{js}"
                        )
                        nc.vector.tensor_copy(stg, pp)
                        nc.sync.dma_start(
                            dst[t * P : (t + 1) * P, js * 512 : (js + 1) * 512], stg
                        )

            # Emit stage A in column panels interleaved with the head batches:
            # heads 0-7 only need QnT/KnT cols 0:512 and Vn rows 0:512, so they
            # can start while the second half of the projections still runs.
            def stageA(part):
                js = part
                gemm(wq_sb, xq_sb, QnT, "q", range(8), (js,))
                gemm(wk_sb, xk_sb, KnT, "k", range(8), (js,))
                gemm(xv_sb, wv_sb, Vn, "v", range(4 * part, 4 * part + 4), (0, 1))

            # ---- Stage B: per-head attention ----
            def headwork(h):
                jk, half = h // 2, h % 2
                # qTf/kTf physical [d, g, r]; logical [d, n'] with n' = r*16+g
                qTf = heads.tile([DH, 16, 64], BF16, tag="qTf", name=f"qTf_{h}")
                kTf = heads.tile([DH, 16, 64], BF16, tag="kTf", name=f"kTf_{h}")
                nc.sync.dma_start(
                    qTf, QnT[:, h * 64 : (h + 1) * 64].rearrange("(g d) r -> d g r", d=DH)
                )
                nc.sync.dma_start(
                    kTf, KnT[:, h * 64 : (h + 1) * 64].rearrange("(g d) r -> d g r", d=DH)
                )
                # v66[p, sk, 1+d] = v_head[sk*128+p, d]; col 65 = ones (rowsum)
                v66 = heads.tile([P, 8, 66], BF16, tag="v66", name=f"v66_{h}")
                nc.sync.dma_start(
                    v66[:, :, 1:65],
                    Vn[h * 64 : (h + 1) * 64, :].rearrange(
                        "(sk p1) (p0 d) -> (p1 p0) sk d", p1=8, d=DH
                    ),
                )
                nc.vector.tensor_copy(v66[:, :, 65:66], ones8)

                # reorder q/k into n'-contiguous [d, 1024]: the stationary
                # operand requires one free dim, and a strided moving operand
                # streams at <1 row/cycle (measured 2.5x slower)
                qTn = heads.tile([DH, N], BF16, tag="qTn", name=f"qTn_{h}")
                nc.vector.tensor_copy(qTn, qTf[:].rearrange("d g r -> d r g"))
                kTn = heads.tile([DH, S], BF16, tag="kTn", name=f"kTn_{h}")
                nc.vector.tensor_copy(kTn, kTf[:].rearrange("d g r -> d r g"))

                po = o_psum.tile([P, 1024], F32, tag="po", name=f"po_{h}")
                for sk in range(8):
                    ps = mm_psum.tile([P, 1024], F32, tag="mm", name=f"ps_{h}_{sk}")
                    for ns in range(2):
                        nc.tensor.matmul(
                            ps[:, ns * 512 : (ns + 1) * 512],
                            kTn[:, P * sk : P * (sk + 1)],
                            qTn[:, ns * 512 : (ns + 1) * 512],
                            start=True,
                            stop=True,
                        )
                    pexp = work.tile([P, 1024], BF16, tag="pexp", name="pexp")
                    nc.scalar.activation(pexp, ps, AF.Exp, scale=SCALE)
                    for ns in range(2):
                        nc.tensor.matmul(
                            po[0:65, ns * 512 : (ns + 1) * 512],
                            v66[:, sk, 1:66],
                            pexp[:, ns * 512 : (ns + 1) * 512],
                            start=(sk == 0),
                            stop=(sk == 7),
                        )

                # softmax divide; rowsum is on psum partition 64
                # softmax denominators: spread the [1, 1024] psum Z row across
                # 128 partitions (DRAM bounce) so the DVE reciprocal runs 128
                # lanes wide instead of 1, then broadcast the reciprocals.
                zsb = sm.tile([65, N], F32, tag="zsb", name=f"zsb_{h}")
                nc.scalar.copy(zsb[64:65, :], po[64:65, :])
                zrow = dram.tile([1, N], F32, tag=f"zrow_{h}", name=f"zrow_{h}")
                nc.sync.dma_start(zrow, zsb[64:65, :])
                zsp = sm.tile([P, 8], F32, tag="zsp", name=f"zsp_{h}")
                nc.sync.dma_start(zsp, zrow[:].rearrange("o (p e) -> (o p) e", p=P))
                rsp = sm.tile([P, 8], F32, tag="rsp", name=f"rsp_{h}")
                nc.vector.reciprocal(rsp, zsp)
                rrow = dram.tile([1, N], F32, tag=f"rrow_{h}", name=f"rrow_{h}")
                nc.sync.dma_start(rrow[:].rearrange("o (p e) -> (o p) e", p=P), rsp)
                rbc = sm.tile([64, N], F32, tag="rbc", name=f"rbc_{h}")
                nc.sync.dma_start(rbc, rrow[:].to_broadcast((64, N)))
                if half == 0:
                    nc.vector.tensor_mul(
                        out=outT[0:64, jk, :], in0=po[0:64, :], in1=rbc
                    )
                else:
                    tmp = sm.tile([64, N], BF16, tag="tmp", name=f"tmp_{h}")
                    nc.vector.tensor_mul(out=tmp, in0=po[0:64, :], in1=rbc)
                    nc.sync.dma_start(outT[64:128, jk, :], tmp)

            for part in range(2):
                stageA(part)
                for h in range(8 * part, 8 * part + 8):
                    headwork(h)

            # ---- Stage C: out = outT.T @ woT + bo ----
            for m in range(8):
                pf = mm_psum.tile([P, 1024], F32, tag="mm", name=f"pf_{m}")
                for isl in range(2):
                    for ck in range(8):
                        nc.tensor.matmul(
                            pf[:, isl * 512 : (isl + 1) * 512],
                            outT[:, ck, m * P : (m + 1) * P],
                            wo_sb[:, ck, isl * 512 : (isl + 1) * 512],
                            start=(ck == 0),
                            stop=(ck == 7),
                        )
                fin = sm.tile([P, 1024], F32, tag="fin", name=f"fin_{m}")
                nc.vector.tensor_add(out=fin, in0=pf, in1=bo_bc)
                nc.sync.dma_start(out[m * P : (m + 1) * P, :], fin)

    nc.compile()
    return nc


_NC_CACHE = {}


def _get_nc():
    if "nc" not in _NC_CACHE:
        _NC_CACHE["nc"] = build()
    return _NC_CACHE["nc"]


TRACE = False


def kernel(queries, keys, values, Wq, Wk, Wv, Wo, bo):
    import ml_dtypes
    from concourse.bass_utils import run_bass_kernel_spmd

    bf16 = ml_dtypes.bfloat16
    qT = [np.ascontiguousarray(np.asarray(queries[i], np.float32).T).astype(bf16) for i in range(B)]
    kT = [np.ascontiguousarray(np.asarray(keys[i], np.float32).T).astype(bf16) for i in range(B)]
    vT = [np.ascontiguousarray(np.asarray(values[i], np.float32).T).astype(bf16) for i in range(B)]
    wqT = np.ascontiguousarray(np.asarray(Wq, np.float32).T).astype(bf16)
    wkT = np.ascontiguousarray(np.asarray(Wk, np.float32).T).astype(bf16)
    wvT = np.ascontiguousarray(np.asarray(Wv, np.float32).T).astype(bf16)
    woT = np.ascontiguousarray(np.asarray(Wo, np.float32).T).astype(bf16)
    bo2 = np.ascontiguousarray(np.asarray(bo, np.float32).reshape(1, DIM))

    nc = _get_nc()
    in_maps = [
        {
            "xqT": qT[i],
            "xkT": kT[i],
            "xvT": vT[i],
            "wqT": wqT,
            "wkT": wkT,
            "wvT": wvT,
            "woT": woT,
            "bo": bo2,
        }
        for i in range(B)
    ]
    res = run_bass_kernel_spmd(nc, in_maps, core_ids=list(range(B)), trace=TRACE)
    if TRACE:
        _NC_CACHE["last_results"] = res
    return np.stack([res.results[i]["out"] for i in range(B)])


# revision 26
# speedup vs baseline: 1.0258x; 1.0258x over previous
"""CrossAttention Trainium2 Bass kernel (v2: bf16, no device transposes).

Problem (hardcoded shapes): B=8, N=S=1024, DIM=1024, H=16, DH=64.
  q = (queries @ Wq.T).reshape(B, H, N, DH)   # direct reshape, NOT a head transpose
  attn = softmax(q @ k^T * DH**-0.5); out = attn @ v
  out = out.transpose(0,2,1,3).reshape(B,N,H*DH) @ Wo.T + bo
Sharding: data-parallel over batch B (one batch element per core, weights
replicated, no collectives).

Host marshalling: inputs arrive transposed and bf16-cast (xqT = queries[i].T,
wqT = Wq.T, ... woT = Wo.T), so the device never runs a PE transpose.

Device per core (X one batch):
  Stage A: QnT = (X@Wq.T).T = Wq@X.T and KnT likewise ([j, n] orientation),
  Vn = X@Wv.T ([n, j]); all bf16, bounced to DRAM so stage B's torch-reshape
  head views become strided (128B-chunk) DMA gathers:
    head h of q as [d, n'] is QnT[g*64+d, h*64+r] with n' = r*16+g
    head h of v as [s', d] is Vn[h*64 + s'//16, (s'%16)*64 + d]
  Stage B per head: scores S_T[s', n'] = kT.T @ qT per 128-row s' block,
  exp on ScalarE (bf16 out), attn@v accumulates po[0:64] with an extra
  all-ones v column producing the softmax denominator on po row 64;
  reciprocal_approx_fast + DRAM-broadcast + tensor_mul divides.
  Stage C: out = outT.T @ woT + bo.
"""

import numpy as np

import concourse.bass as bass
import concourse.mybir as mybir
import concourse.tile as tile
from concourse import bacc

B, N, S, DIM, H, DH = 8, 1024, 1024, 1024, 16, 64
SCALE = DH**-0.5
P = 128
F32 = mybir.dt.float32
BF16 = mybir.dt.bfloat16
AF = mybir.ActivationFunctionType


def build(debug: bool = False) -> bacc.Bacc:
    nc = bacc.Bacc("TRN2", target_bir_lowering=False, debug=debug, num_devices=B)

    xqT = nc.dram_tensor("xqT", [DIM, N], BF16, kind="ExternalInput")
    xkT = nc.dram_tensor("xkT", [DIM, S], BF16, kind="ExternalInput")
    xvT = nc.dram_tensor("xvT", [DIM, S], BF16, kind="ExternalInput")
    wqT = nc.dram_tensor("wqT", [DIM, H * DH], BF16, kind="ExternalInput")
    wkT = nc.dram_tensor("wkT", [DIM, H * DH], BF16, kind="ExternalInput")
    wvT = nc.dram_tensor("wvT", [DIM, H * DH], BF16, kind="ExternalInput")
    woT = nc.dram_tensor("woT", [H * DH, DIM], BF16, kind="ExternalInput")
    bo = nc.dram_tensor("bo", [1, DIM], F32, kind="ExternalInput")
    out = nc.dram_tensor("out", [N, DIM], F32, kind="ExternalOutput")

    with tile.TileContext(nc) as tc:
        with (
            tc.tile_pool(name="const", bufs=1) as const,
            tc.tile_pool(name="persist", bufs=1) as persist,
            tc.tile_pool(name="stg", bufs=3) as stg_pool,
            tc.tile_pool(name="heads", bufs=2) as heads,
            tc.tile_pool(name="work", bufs=3) as work,
            tc.tile_pool(name="sm", bufs=2) as sm,
            tc.tile_pool(name="mm_psum", bufs=2, space="PSUM") as mm_psum,
            tc.tile_pool(name="o_psum", bufs=2, space="PSUM") as o_psum,
            tc.tile_pool(name="dram", bufs=1, space="DRAM") as dram,
        ):
            ones8 = const.tile([P, 8, 1], BF16)
            nc.vector.memset(ones8, 1.0)
            bo_bc = persist.tile([P, DIM], F32)
            nc.sync.dma_start(bo_bc, bo[:].to_broadcast((P, DIM)))

            # SBUF-resident operands, [c (or j), ck, free] with c = ck*128 + p
            xq_sb = persist.tile([P, 8, N], BF16, name="xq_sb")
            xk_sb = persist.tile([P, 8, S], BF16, name="xk_sb")
            xv_sb = persist.tile([P, 8, S], BF16, name="xv_sb")
            wq_sb = persist.tile([P, 8, H * DH], BF16, name="wq_sb")
            wk_sb = persist.tile([P, 8, H * DH], BF16, name="wk_sb")
            wv_sb = persist.tile([P, 8, H * DH], BF16, name="wv_sb")
            wo_sb = persist.tile([P, 8, DIM], BF16, name="wo_sb")
            outT = persist.tile([P, 8, N], BF16, name="outT")

            for src, dst in (
                (xqT, xq_sb), (xkT, xk_sb), (xvT, xv_sb),
                (wqT, wq_sb), (wkT, wk_sb), (wvT, wv_sb), (woT, wo_sb),
            ):
                v = src[:].rearrange("(ck c) f -> c ck f", c=P)
                for i in range(4):
                    nc.sync.dma_start(dst[:, 2 * i : 2 * i + 2, :], v[:, 2 * i : 2 * i + 2, :])

            QnT = dram.tile([H * DH, N], BF16, name="QnT")
            KnT = dram.tile([H * DH, S], BF16, name="KnT")
            Vn = dram.tile([S, H * DH], BF16, name="Vn")

            # ---- Stage A: projections (no transposes; operands pre-transposed) ----
            # QnT[j, n] = sum_c Wq[j, c] X[n, c] : lhsT = wq_sb (stationary), rhs = xq_sb
            # Vn[n, j]  = sum_c X[n, c] Wv[j, c] : lhsT = xv_sb, rhs = wv_sb
            def gemm(lhs_sb, rhs_sb, dst, tag, trange, jsrange):
                for t in trange:
                    for js in jsrange:
                        pp = mm_psum.tile(
                            [P, 512], F32, tag="mm", name=f"pp_{tag}_{t}_{js}"
                        )
                        for ck in range(8):
                            nc.tensor.matmul(
                                pp,
                                lhs_sb[:, ck, t * P : (t + 1) * P],
                                rhs_sb[:, ck, js * 512 : (js + 1) * 512],
                                start=(ck == 0),
                                stop=(ck == 7),
                            )
                        stg = stg_pool.tile(
                            [P, 512], BF16, tag="stg", name=f"stg_{tag}_{t}_{js}"
                        )
                        nc.vector.tensor_copy(stg, pp)
                        nc.sync.dma_start(
                            dst[t * P : (t + 1) * P, js * 512 : (js + 1) * 512], stg
                        )

            # Emit stage A in column panels interleaved with the head batches:
            # heads 0-7 only need QnT/KnT cols 0:512 and Vn rows 0:512, so they
            # can start while the second half of the projections still runs.
            def stageA(part):
                js = part
                gemm(wq_sb, xq_sb, QnT, "q", range(8), (js,))
                gemm(wk_sb, xk_sb, KnT, "k", range(8), (js,))
                gemm(xv_sb, wv_sb, Vn, "v", range(4 * part, 4 * part + 4), (0, 1))

            # ---- Stage B: per-head attention ----
            def headwork(h):
                jk, half = h // 2, h % 2
                # qTf/kTf physical [d, g, r]; logical [d, n'] with n' = r*16+g
                qTf = heads.tile([DH, 16, 64], BF16, tag="qTf", name=f"qTf_{h}")
                kTf = heads.tile([DH, 16, 64], BF16, tag="kTf", name=f"kTf_{h}")
                nc.sync.dma_start(
                    qTf, QnT[:, h * 64 : (h + 1) * 64].rearrange("(g d) r -> d g r", d=DH)
                )
                nc.sync.dma_start(
                    kTf, KnT[:, h * 64 : (h + 1) * 64].rearrange("(g d) r -> d g r", d=DH)
                )
                # v66[p, sk, 1+d] = v_head[sk*128+p, d]; col 65 = ones (rowsum)
                v66 = heads.tile([P, 8, 66], BF16, tag="v66", name=f"v66_{h}")
                nc.sync.dma_start(
                    v66[:, :, 1:65],
                    Vn[h * 64 : (h + 1) * 64, :].rearrange(
                        "(sk p1) (p0 d) -> (p1 p0) sk d", p1=8, d=DH
                    ),
                )
                nc.vector.tensor_copy(v66[:, :, 65:66], ones8)

                # reorder q/k into n'-contiguous [d, 1024]: the stationary
                # operand requires one free dim, and a strided moving operand
                # streams at <1 row/cycle (measured 2.5x slower)
                qTn = heads.tile([DH, N], BF16, tag="qTn", name=f"qTn_{h}")
                nc.vector.tensor_copy(qTn, qTf[:].rearrange("d g r -> d r g"))
                kTn = heads.tile([DH, S], BF16, tag="kTn", name=f"kTn_{h}")
                nc.vector.tensor_copy(kTn, kTf[:].rearrange("d g r -> d r g"))

                po = o_psum.tile([P, 1024], F32, tag="po", name=f"po_{h}")
                for sk in range(8):
                    ps = mm_psum.tile([P, 1024], F32, tag="mm", name=f"ps_{h}_{sk}")
                    for ns in range(2):
                        nc.tensor.matmul(
                            ps[:, ns * 512 : (ns + 1) * 512],
                            kTn[:, P * sk : P * (sk + 1)],
                            qTn[:, ns * 512 : (ns + 1) * 512],
                            start=True,
                            stop=True,
                        )
                    pexp = work.tile([P, 1024], BF16, tag="pexp", name="pexp")
                    nc.scalar.activation(pexp, ps, AF.Exp, scale=SCALE)
                    for ns in range(2):
                        nc.tensor.matmul(
                            po[0:65, ns * 512 : (ns + 1) * 512],
                            v66[:, sk, 1:66],
                            pexp[:, ns * 512 : (ns + 1) * 512],
                            start=(sk == 0),
                            stop=(sk == 7),
                        )

                # softmax divide; rowsum is on psum partition 64
                # softmax denominators: spread the [1, 1024] psum Z row across
                # 128 partitions (DRAM bounce) so the DVE reciprocal runs 128
                # lanes wide instead of 1, then broadcast the reciprocals.
                zsb = sm.tile([65, N], F32, tag="zsb", name=f"zsb_{h}")
                nc.scalar.copy(zsb[64:65, :], po[64:65, :])
                zrow = dram.tile([1, N], F32, tag=f"zrow_{h}", name=f"zrow_{h}")
                nc.sync.dma_start(zrow, zsb[64:65, :])
                zsp = sm.tile([P, 8], F32, tag="zsp", name=f"zsp_{h}")
                nc.sync.dma_start(zsp, zrow[:].rearrange("o (p e) -> (o p) e", p=P))
                rsp = sm.tile([P, 8], F32, tag="rsp", name=f"rsp_{h}")
                nc.vector.reciprocal(rsp, zsp)
                rrow = dram.tile([1, N], F32, tag=f"rrow_{h}", name=f"rrow_{h}")
                nc.sync.dma_start(rrow[:].rearrange("o (p e) -> (o p) e", p=P), rsp)
                rbc = sm.tile([64, N], F32, tag="rbc", name=f"rbc_{h}")
                nc.sync.dma_start(rbc, rrow[:].to_broadcast((64, N)))
                if half == 0:
                    nc.vector.tensor_mul(
                        out=outT[0:64, jk, :], in0=po[0:64, :], in1=rbc
                    )
                else:
                    tmp = sm.tile([64, N], BF16, tag="tmp", name=f"tmp_{h}")
                    nc.vector.tensor_mul(out=tmp, in0=po[0:64, :], in1=rbc)
                    nc.sync.dma_start(outT[64:128, jk, :], tmp)

            stageA(0)
            stageA(1)
            for h in range(H):
                headwork(h)

            # ---- Stage C: out = outT.T @ woT + bo ----
            for m in range(8):
                pf = mm_psum.tile([P, 1024], F32, tag="mm", name=f"pf_{m}")
                for isl in range(2):
                    for ck in range(8):
                        nc.tensor.matmul(
                            pf[:, isl * 512 : (isl + 1) * 512],
                            outT[:, ck, m * P : (m + 1) * P],
                            wo_sb[:, ck, isl * 512 : (isl + 1) * 512],
                            start=(ck == 0),
                            stop=(ck == 7),
                        )
                fin = sm.tile([P, 1024], F32, tag="fin", name=f"fin_{m}")
                nc.vector.tensor_add(out=fin, in0=pf, in1=bo_bc)
                nc.sync.dma_start(out[m * P : (m + 1) * P, :], fin)

    nc.compile()
    return nc


_NC_CACHE = {}


def _get_nc():
    if "nc" not in _NC_CACHE:
        _NC_CACHE["nc"] = build()
    return _NC_CACHE["nc"]


TRACE = False


def kernel(queries, keys, values, Wq, Wk, Wv, Wo, bo):
    import ml_dtypes
    from concourse.bass_utils import run_bass_kernel_spmd

    bf16 = ml_dtypes.bfloat16
    qT = [np.ascontiguousarray(np.asarray(queries[i], np.float32).T).astype(bf16) for i in range(B)]
    kT = [np.ascontiguousarray(np.asarray(keys[i], np.float32).T).astype(bf16) for i in range(B)]
    vT = [np.ascontiguousarray(np.asarray(values[i], np.float32).T).astype(bf16) for i in range(B)]
    wqT = np.ascontiguousarray(np.asarray(Wq, np.float32).T).astype(bf16)
    wkT = np.ascontiguousarray(np.asarray(Wk, np.float32).T).astype(bf16)
    wvT = np.ascontiguousarray(np.asarray(Wv, np.float32).T).astype(bf16)
    woT = np.ascontiguousarray(np.asarray(Wo, np.float32).T).astype(bf16)
    bo2 = np.ascontiguousarray(np.asarray(bo, np.float32).reshape(1, DIM))

    nc = _get_nc()
    in_maps = [
        {
            "xqT": qT[i],
            "xkT": kT[i],
            "xvT": vT[i],
            "wqT": wqT,
            "wkT": wkT,
            "wvT": wvT,
            "woT": woT,
            "bo": bo2,
        }
        for i in range(B)
    ]
    res = run_bass_kernel_spmd(nc, in_maps, core_ids=list(range(B)), trace=TRACE)
    if TRACE:
        _NC_CACHE["last_results"] = res
    return np.stack([res.results[i]["out"] for i in range(B)])
